# revision 20
# baseline (speedup 1.0000x reference)
"""Trainium2 Bass kernel for nn_EstraNet_1443109012284.

Mathematical reduction: the reference's FAVOR+/trig branch (phi_q, aux_q/k,
fr_q/k, aux_A, A) does not feed the output.  The output is exactly

    out[b,n,d] = sum_{h,c} W_o[h,c,d] * norma[h] * sum_{d'} W_v[d',h,c] * x[b,n,d']
               = (x @ M)[b,n,d],   M[d',d] = sum_{h,c} W_v[d',h,c] norma[h] W_o[h,c,d]

with norma[h] = || sum_d s_p[h] W_p[d,h,:] beta_p[d] ||_2.

M is a tiny [512,512] matrix folded on the host; the device does the single
big GEMM  y[32768,512] = x[32768,512] @ M[512,512]  data-parallel over rows:
each of the 8 cores handles 4096 rows (yT[d,n] = sum_k M[k,d] xT[k,n]).

Default path is fp16 ("fp16v2").  fp8 was implemented (fp8x3 path, kept
below for reference) and MEASURED SLOWER: on TRN2 hardware a DoubleRow
e4m3 matmul streams 512 pair-columns in the same 216ns as a 512-col fp16
matmul — i.e. fp8 is 2x FLOPs/instruction via the doubled (256-deep)
contraction, NOT the 4x the cost model's 0.5 cycles/row suggests.  Since
plain e4m3 fails the max|err|/max|y| < 2e-2 gate (measured 3.9e-2; one-
sided compensation 2.7e-2), a passing fp8 scheme needs >= 3 GEMM
equivalents (x_hi@M_hi + x_lo@M_hi + x_hi@M_lo, measured 1.2e-3) which is
1.5x the fp16 PE time (41.5us vs 27.7us stream; fp8x3 measured 58.3us
total vs ~46us fp16).  The fp16 128 x 216ns = 27.7us PE stream is the
floor for this metric.

Exec-time anatomy at ~46us (trace-measured, all per-launch fixed costs
are INSIDE the measured window):
  ~5.3us  wait for first data  (first DMA packet ~2.3-2.7us after kernel
          start; pipe-fill is aggregate-HBM-bound at ~220-300 B/ns shared
          across all rings/cores, NOT per-ring; receipt sem ~0.7-2us after
          last byte, load-dependent)
  27.7us  PE stream (gapless when tuned)
  ~3.2us  tail: last psum drain + store + receipt
  ~9.5us  teardown: output-receipt waits + a compiler-injected sweep that
          zeroes all 255 semaphores one-by-one across the 5 engines
          (~6.5us; emitted by the NEFF backend, not by this program)

Schedule rules learned the hard way:
- A PE idle gap >~1us re-throttles the HAM clock to half rate for ~13
  matmuls (427ns spacing, ~2.7us lost).  Gaps <= ~0.9us do not.  So the
  N_WARM warmup matmuls must bridge from engine-start to the first
  data-gated matmul with no gap (40 x ~107ns fits the ~4.3us window).
- Tile dep-tracking is whole-tile: a matmul reading cols [0:512] of a
  tile still waits for a DMA writing cols [512:1024].  Splitting hot
  tiles into per-j tiles did not help in practice (gates are receipt-
  latency-bound, not transfer-bound) and H0_SPLIT measured worse.
- Only SP (sync) and Activation (scalar) have HWDGE rings; gpsimd drives
  the slower SWDGE.  Keeping the sync ring light is what kills the
  47-50us outlier runs: m1/m23/x03 ride gpsimd (KERNEL_X03_G=1), which
  is otherwise idle until outputs start ~13us in.  Measured effect:
  8-rep spread tightened from [45.5..48.6]us to [45.5..46.5]us.
- x chunks alternate sync/scalar per k in consumption order; m[k0] on
  sync and x[h0,k0] on scalar lead the rings with one receipt each.
- PSUM->SBUF drains split by bank parity: ACT copies j0 banks, DVE j1;
- outputs blocked [h,d,128,1024] fp16; early stores on gpsimd SWDGE,
  later ones on sync/scalar; final tile's two halves store on sync and
  scalar in parallel; last quarter runs d-outer to spread its stores.
"""

import os as _os
import sys

sys.path.insert(0, "/opt/trn_rl_repo")

import numpy as np

import concourse.bass as bass
import concourse.tile as tile
from concourse import bacc, mybir
from concourse.bass_utils import run_bass_kernel_spmd

N_CORES = 8
ROWS = 32768           # B*N = 8*4096
RPC = ROWS // N_CORES  # rows per core = 4096
D = 512
DT = D // 128          # output row-blocks = 4
HB = 4                 # n-quarters per stripe
HW = RPC // HB         # 1024 columns per quarter
JH = HW // 512         # moving chunks of 512 per quarter = 2

# fp16 path constants
KC = 4                 # fp16 contraction chunks of 128
# fp8 DoubleRow path constants
S = 2                  # DoubleRow k-steps (256-deep contraction each)
I = 2                  # interleaved k-rows per partition per step

COMPUTE_DTYPE = _os.environ.get("KERNEL_DTYPE", "fp16v2")
N_WARM = int(_os.environ.get("KERNEL_NWARM", "40"))
WARM_MEMSET = _os.environ.get("KERNEL_WARM_MEMSET", "1") == "1"
H0_SPLIT = _os.environ.get("KERNEL_H0_SPLIT", "0") == "1"

_DT = {
    "fp32": mybir.dt.float32,
    "f32r": mybir.dt.float32r,
    "bf16": mybir.dt.bfloat16,
    "fp16": mybir.dt.float16,
}


def _np_dtype(token):
    if token == "bf16":
        import ml_dtypes

        return ml_dtypes.bfloat16
    if token == "fp16":
        return np.float16
    return np.float32


def _f8np():
    import ml_dtypes

    return ml_dtypes.float8_e4m3


def _build_fp8x3():
    f8 = mybir.dt.float8e4
    dt_out = mybir.dt.float16
    DR = mybir.MatmulPerfMode.DoubleRow
    nc = bacc.Bacc("TRN2", target_bir_lowering=False)
    # x_hi/x_lo for quarter 0, split by DoubleRow step s for fine DMA chunks
    xh0 = nc.dram_tensor("xh0", [S, 128, I * HW], f8, kind="ExternalInput")
    xl0 = nc.dram_tensor("xl0", [S, 128, I * HW], f8, kind="ExternalInput")
    # quarters h1..h3
    xhq = nc.dram_tensor("xhq", [HB - 1, 128, S * I * HW], f8, kind="ExternalInput")
    xlq = nc.dram_tensor("xlq", [HB - 1, 128, S * I * HW], f8, kind="ExternalInput")
    mh = nc.dram_tensor("mh", [128, S * I * D], f8, kind="ExternalInput")
    ml = nc.dram_tensor("ml", [128, S * I * D], f8, kind="ExternalInput")
    # output blocked [h, d, 128, 1024] so each store is DRAM-contiguous
    yt = nc.dram_tensor("yt", [HB, DT, 128, HW], dt_out, kind="ExternalOutput")

    with tile.TileContext(nc) as tc:
        with (
            tc.tile_pool(name="xp", bufs=1) as xp,
            tc.tile_pool(name="mp", bufs=1) as mp,
            tc.tile_pool(name="op", bufs=4) as op,
            tc.tile_pool(name="pp", bufs=8, space="PSUM") as pp,
        ):
            # PE warmup (see module docstring)
            wz = mp.tile([128, 128], mybir.dt.float16, name="wz")
            if WARM_MEMSET:
                nc.gpsimd.memset(wz[:], 1.0)
            warm = pp.tile([128, 512], mybir.dt.float32, tag="ps", name="warm")
            for w in range(N_WARM):
                nc.tensor.matmul(
                    warm[:, 0:128], wz[:], wz[:], start=True, stop=True
                )

            m_hi = mp.tile([128, S, I, D], f8, name="m_hi")
            m_lo = mp.tile([128, S, I, D], f8, name="m_lo")
            x_sb = {}

            def xtile(h, t):
                tl = xp.tile([128, S, I, HW], f8, tag=f"x{t}{h}", name=f"x{t}{h}")
                x_sb[(h, t)] = tl
                return tl

            # leading pair: m_hi(s0) on sync, x_hi[h0,s0] on scalar
            th0 = xtile(0, 0)
            tl0 = xtile(0, 1)
            nc.sync.dma_start(out=m_hi[:, 0], in_=mh[:, : I * D])
            nc.scalar.dma_start(out=th0[:, 0], in_=xh0[0])
            nc.sync.dma_start(out=th0[:, 1], in_=xh0[1])
            nc.scalar.dma_start(out=m_hi[:, 1], in_=mh[:, I * D :])
            nc.sync.dma_start(out=tl0[:, 0], in_=xl0[0])
            nc.scalar.dma_start(out=tl0[:, 1], in_=xl0[1])
            nc.sync.dma_start(out=m_lo[:, 0], in_=ml[:, : I * D])
            nc.scalar.dma_start(out=m_lo[:, 1], in_=ml[:, I * D :])
            for h in range(1, HB):
                for t in range(2):           # 0 = x_hi, 1 = x_lo
                    src = xhq if t == 0 else xlq
                    tl = xtile(h, t)
                    for s in range(S):
                        eng = nc.scalar if (t * S + s) % 2 == 0 else nc.sync
                        eng.dma_start(
                            out=tl[:, s],
                            in_=src[h - 1, :, s * I * HW : (s + 1) * I * HW],
                        )

            # output engine per (h,d) tile: gpsimd early (HWDGE rings still
            # pulling inputs), rotate later so no ring serializes
            G, Sy, C = nc.gpsimd, nc.sync, nc.scalar
            OENG = [
                G, G, G, G,
                G, G, Sy, C,
                Sy, C, G, C,
                G, G, Sy, None,  # last tile handled fine-grained below
            ]

            def copy_eng(j):
                # one PSUM reader per bank: ACT drains j0 banks, DVE j1 banks
                return nc.scalar.copy if j == 0 else nc.vector.tensor_copy

            # term-step sequence: (x-operand, m-operand, k-step)
            TS = [(0, m_hi, 0), (0, m_hi, 1),
                  (1, m_hi, 0), (1, m_hi, 1),
                  (0, m_lo, 0), (0, m_lo, 1)]

            def emit_mm(pss, h, ts, d, j):
                t, m_t, s = TS[ts]
                nc.tensor.matmul(
                    pss[d * JH + j][:],
                    m_t[:, s, :, d * 128 : (d + 1) * 128],
                    x_sb[(h, t)][:, s, :, j * 512 : (j + 1) * 512],
                    start=(ts == 0),
                    stop=(ts == len(TS) - 1),
                    perf_mode=DR,
                )

            def emit_out(pss, h, d):
                ot = op.tile([128, HW], dt_out, name=f"ot{h}{d}", tag="ot")
                last = h == HB - 1 and d == DT - 1
                if last:
                    # final tile: j0 bank (ACT -> sync) and j1 bank (DVE ->
                    # scalar) drain on parallel engines and rings
                    for j in range(JH):
                        c0 = j * 512
                        copy_eng(j)(ot[:, c0 : c0 + 512],
                                    pss[d * JH + j][:])
                        seng = nc.sync if j == 0 else nc.scalar
                        seng.dma_start(
                            out=yt[h, d, :, c0 : c0 + 512],
                            in_=ot[:, c0 : c0 + 512],
                        )
                else:
                    for j in range(JH):
                        copy_eng(j)(
                            ot[:, j * 512 : (j + 1) * 512],
                            pss[d * JH + j][:],
                        )
                    OENG[h * DT + d].dma_start(out=yt[h, d], in_=ot[:])

            for h in range(HB):
                pss = [
                    pp.tile([128, 512], mybir.dt.float32, tag="ps",
                            name=f"ps_{h}_{dj // JH}_{dj % JH}")
                    for dj in range(DT * JH)
                ]
                if h < HB - 1:
                    # ts-outer: quarter starts after just its first chunk
                    for ts in range(len(TS)):
                        for d in range(DT):
                            for j in range(JH):
                                emit_mm(pss, h, ts, d, j)
                    for d in range(DT):
                        emit_out(pss, h, d)
                else:
                    # last quarter d-outer: inputs all resident; spreads the
                    # final 1MB of output instead of bunching it at the end
                    for d in range(DT):
                        for ts in range(len(TS)):
                            for j in range(JH):
                                emit_mm(pss, h, ts, d, j)
                        emit_out(pss, h, d)
    nc.compile()
    return nc


def _build_fp16(token):
    v2 = token.endswith("v2")
    if v2:
        token = token[:-2]
    dt_in = _DT[token]
    dt_out = mybir.dt.float16 if token == "fp16" else mybir.dt.float32
    nc = bacc.Bacc("TRN2", target_bir_lowering=False)
    xt0 = nc.dram_tensor("xt0", [KC, 128, HW], dt_in, kind="ExternalInput")
    xq = nc.dram_tensor("xq", [HB - 1, 128, KC * HW], dt_in, kind="ExternalInput")
    mm = nc.dram_tensor("mm", [128, KC, D], dt_in, kind="ExternalInput")
    yt = nc.dram_tensor("yt", [HB, DT, 128, HW], dt_out, kind="ExternalOutput")

    with tile.TileContext(nc) as tc:
        with (
            tc.tile_pool(name="xp", bufs=1) as xp,
            tc.tile_pool(name="mp", bufs=1) as mp,
            tc.tile_pool(name="op", bufs=4) as op,
            tc.tile_pool(name="pp", bufs=8, space="PSUM") as pp,
        ):
            wz = mp.tile([128, 128], mybir.dt.float16, name="wz")
            if WARM_MEMSET:
                nc.gpsimd.memset(wz[:], 1.0)
            warm = pp.tile([128, 512], mybir.dt.float32, tag="ps", name="warm")
            for w in range(N_WARM):
                nc.tensor.matmul(
                    warm[:, 0:128], wz[:], wz[:], start=True, stop=True
                )

            m_sb = mp.tile([128, KC, D], dt_in, name="m_sb")
            x_sb = {}

            def xtile(h, k):
                t = xp.tile([128, HW], dt_in, tag=f"x{h}{k}", name=f"x{h}{k}")
                x_sb[(h, k)] = t
                return t

            if v2:
                # three input rings for pipe-fill (only SP/ACT have HWDGE;
                # gpsimd SWDGE is idle until outputs start ~13us in, so it
                # takes the m tail off the hot rings).  Pipe-fill is
                # aggregate-HBM-bound (~220-300 B/ns across rings), so the
                # goal is ordering, not fan-out: m0+x00 race first, x01
                # right behind m0 so the k1 group isn't receipt-stalled.
                # h0's hot tiles (k0, k1) are split into per-j tiles so a
                # matmul only gates on its own 128KB half-chunk's receipt
                # (tile dep-tracking is whole-tile).
                def xjtile(k, jj):
                    tl = xp.tile([128, 512], dt_in, tag=f"xj{k}{jj}",
                                 name=f"xj{k}{jj}")
                    x_sb[(0, k, jj)] = tl
                    return tl

                if H0_SPLIT:
                    nc.sync.dma_start(out=m_sb[:, 0, :], in_=mm[:, 0, :])
                    nc.scalar.dma_start(out=xjtile(0, 0)[:], in_=xt0[0, :, 0:512])
                    nc.gpsimd.dma_start(out=m_sb[:, 1, :], in_=mm[:, 1, :])
                    nc.sync.dma_start(out=xjtile(1, 0)[:], in_=xt0[1, :, 0:512])
                    nc.scalar.dma_start(out=xjtile(0, 1)[:], in_=xt0[0, :, 512:])
                    nc.sync.dma_start(out=xjtile(1, 1)[:], in_=xt0[1, :, 512:])
                    nc.scalar.dma_start(out=xtile(0, 2)[:], in_=xt0[2])
                    nc.gpsimd.dma_start(out=m_sb[:, 2:KC, :], in_=mm[:, 2:KC, :])
                    nc.sync.dma_start(out=xtile(0, 3)[:], in_=xt0[3])
                else:
                    x03_eng = nc.gpsimd if _os.environ.get(
                        "KERNEL_X03_G", "1") == "1" else nc.sync
                    x00_2q = _os.environ.get("KERNEL_X00_2Q", "0") == "1"
                    t00 = xtile(0, 0)
                    nc.sync.dma_start(out=m_sb[:, 0, :], in_=mm[:, 0, :])
                    if x00_2q:
                        # first tile raced in halves on both rings: the
                        # first matmul gates on two parallel 128KB DMAs
                        nc.scalar.dma_start(out=t00[:, 0:512],
                                            in_=xt0[0, :, 0:512])
                        nc.sync.dma_start(out=t00[:, 512:],
                                          in_=xt0[0, :, 512:])
                    else:
                        nc.scalar.dma_start(out=t00[:], in_=xt0[0])
                    nc.gpsimd.dma_start(out=m_sb[:, 1, :], in_=mm[:, 1, :])
                    nc.sync.dma_start(out=xtile(0, 1)[:], in_=xt0[1])
                    nc.scalar.dma_start(out=xtile(0, 2)[:], in_=xt0[2])
                    nc.gpsimd.dma_start(out=m_sb[:, 2:KC, :], in_=mm[:, 2:KC, :])
                    x03_eng.dma_start(out=xtile(0, 3)[:], in_=xt0[3])
                for h in range(1, HB):
                    for k in range(KC):
                        eng = nc.scalar if k % 2 == 0 else nc.sync
                        eng.dma_start(
                            out=xtile(h, k)[:],
                            in_=xq[h - 1, :, k * HW : (k + 1) * HW],
                        )
            else:
                nc.sync.dma_start(out=m_sb[:, 0, :], in_=mm[:, 0, :])
                nc.scalar.dma_start(out=xtile(0, 0)[:], in_=xt0[0])
                nc.sync.dma_start(out=m_sb[:, 1, :], in_=mm[:, 1, :])
                nc.scalar.dma_start(out=m_sb[:, 2:KC, :], in_=mm[:, 2:KC, :])
                nc.sync.dma_start(out=xtile(0, 1)[:], in_=xt0[1])
                nc.scalar.dma_start(out=xtile(0, 2)[:], in_=xt0[2])
                nc.sync.dma_start(out=xtile(0, 3)[:], in_=xt0[3])
                for h in range(1, HB):
                    for k in range(KC):
                        eng = nc.scalar if k % 2 == 0 else nc.sync
                        eng.dma_start(
                            out=xtile(h, k)[:],
                            in_=xq[h - 1, :, k * HW : (k + 1) * HW],
                        )

            G, Sy, C = nc.gpsimd, nc.sync, nc.scalar
            OENG = [
                G, G, G, G,
                G, G, Sy, C,
                Sy, C, G, C,
                G, G, Sy, None,
            ]

            def copy_eng(j):
                return nc.scalar.copy if j == 0 else nc.vector.tensor_copy

            def emit_mm(pss, h, k, d, j):
                if (h, k, j) in x_sb:
                    rhs = x_sb[(h, k, j)][:]
                else:
                    rhs = x_sb[(h, k)][:, j * 512 : (j + 1) * 512]
                nc.tensor.matmul(
                    pss[d * JH + j][:],
                    m_sb[:, k, d * 128 : (d + 1) * 128],
                    rhs,
                    start=(k == 0),
                    stop=(k == KC - 1),
                )

            def emit_out(pss, h, d):
                ot = op.tile([128, HW], dt_out, name=f"ot{h}{d}", tag="ot")
                last = h == HB - 1 and d == DT - 1
                if last:
                    for j in range(JH):
                        c0 = j * 512
                        copy_eng(j)(ot[:, c0 : c0 + 512],
                                    pss[d * JH + j][:])
                        seng = nc.sync if j == 0 else nc.scalar
                        seng.dma_start(
                            out=yt[h, d, :, c0 : c0 + 512],
                            in_=ot[:, c0 : c0 + 512],
                        )
                else:
                    for j in range(JH):
                        copy_eng(j)(
                            ot[:, j * 512 : (j + 1) * 512],
                            pss[d * JH + j][:],
                        )
                    OENG[h * DT + d].dma_start(out=yt[h, d], in_=ot[:])

            for h in range(HB):
                pss = [
                    pp.tile([128, 512], mybir.dt.float32, tag="ps",
                            name=f"ps_{h}_{dj // JH}_{dj % JH}")
                    for dj in range(DT * JH)
                ]
                if h < HB - 1:
                    for k in range(KC):
                        if v2 and H0_SPLIT and h == 0 and k < 2:
                            # split tiles: run all j0 mms before j1's 128KB
                            # half-chunk is needed
                            for j in range(JH):
                                for d in range(DT):
                                    emit_mm(pss, h, k, d, j)
                        else:
                            for d in range(DT):
                                for j in range(JH):
                                    emit_mm(pss, h, k, d, j)
                    for d in range(DT):
                        emit_out(pss, h, d)
                else:
                    for d in range(DT):
                        for k in range(KC):
                            for j in range(JH):
                                emit_mm(pss, h, k, d, j)
                        emit_out(pss, h, d)
    nc.compile()
    return nc


def _build(token):
    if token == "fp8x3":
        return _build_fp8x3()
    return _build_fp16(token)


def _fold_m(W_v, s_p, W_p, beta_p, W_o):
    """Host-side constant folding of the tiny parameter tensors into M."""
    W_v = np.asarray(W_v, dtype=np.float64)
    s_p = np.asarray(s_p, dtype=np.float64)
    W_p = np.asarray(W_p, dtype=np.float64)
    beta_p = np.asarray(beta_p, dtype=np.float64)
    W_o = np.asarray(W_o, dtype=np.float64)
    phi = np.einsum("h,dhc,d->hc", s_p, W_p, beta_p)
    norma = np.linalg.norm(phi, axis=1)  # [h]
    M = np.einsum("dhc,h,hce->de", W_v, norma, W_o)  # [512, 512]
    return M.astype(np.float32)


def _k_layout(a, ncols_per_row):
    """[512, C] array -> [128, S*I*C'] rows with k = 256*s + 128*i + p."""
    C = a.shape[1]
    return np.ascontiguousarray(
        a.reshape(S, I, 128, C).transpose(2, 0, 1, 3).reshape(128, S * I * C)
    )


def _prep_fp8x3(x, M):
    """Quantize + lay out all device inputs for the fp8x3 path."""
    f8 = _f8np()
    # output/range scale: put y*32*sM peak near 2^14..2^15 in fp16
    col_norm_max = float(np.linalg.norm(M, axis=0).max())
    ymax_est = 7.5 * col_norm_max  # 5.77 sigma for 16.7M gaussians + margin
    e = int(np.floor(np.log2(32768.0 / (32.0 * ymax_est))))
    # fp8 range guard for M*sM (e4m3 max 240; keep <= 224)
    m_amax = float(np.abs(M).max())
    e = min(e, int(np.floor(np.log2(224.0 / m_amax))))
    sM = np.float32(2.0 ** e)

    Ms = (M * sM).astype(np.float32)
    M_hi8 = Ms.astype(f8)
    M_lo8 = (Ms - M_hi8.astype(np.float32)).astype(f8)
    mh = _k_layout(M_hi8.astype(np.float32), D).astype(f8)
    ml = _k_layout(M_lo8.astype(np.float32), D).astype(f8)

    out_unscale = np.float32(1.0 / (32.0 * float(sM)))

    in_maps = []
    xf = x.reshape(ROWS, D)
    for c in range(N_CORES):
        x32 = (xf[c * RPC : (c + 1) * RPC].T * np.float32(32.0)).astype(np.float32)
        xh8 = x32.astype(f8)                                # [512, 4096]
        xl8 = (x32 - xh8.astype(np.float32)).astype(f8)

        def blobs(a8):
            ar = a8.reshape(S, I, 128, HB, HW)              # [s,i,p,h,c]
            b0 = np.ascontiguousarray(
                ar[:, :, :, 0, :].transpose(0, 2, 1, 3).reshape(S, 128, I * HW)
            )
            bq = np.ascontiguousarray(
                ar[:, :, :, 1:, :].transpose(3, 2, 0, 1, 4).reshape(
                    HB - 1, 128, S * I * HW
                )
            )
            return b0, bq

        xh0, xhq = blobs(xh8)
        xl0, xlq = blobs(xl8)
        in_maps.append(
            {"xh0": xh0, "xhq": xhq, "xl0": xl0, "xlq": xlq, "mh": mh, "ml": ml}
        )
    return in_maps, out_unscale


_prog_cache = {}
_last_in_maps = None  # kept for test.py profiling reuse
_last_result = None


def _run(in_maps, token, **kwargs):
    if token not in _prog_cache:
        _prog_cache[token] = _build(token)
    return run_bass_kernel_spmd(_prog_cache[token], in_maps, list(range(N_CORES)), **kwargs)


def kernel(x, W_v, s_p, c_p, W_p, W_A, W_o, beta_p, beta_i_p, **_unused):
    global _last_in_maps, _last_result
    token = COMPUTE_DTYPE

    x = np.asarray(x, dtype=np.float32)
    M = _fold_m(W_v, s_p, W_p, beta_p, W_o)
    B, N, Dd = x.shape
    assert B * N == ROWS and Dd == D, (x.shape,)

    if token == "fp8x3":
        in_maps, out_unscale = _prep_fp8x3(x, M)
    else:
        base = token[:-2] if token.endswith("v2") else token
        np_dt = _np_dtype(base)
        # fp16 path: scale M by an exact power of two so M entries and y
        # values sit in fp16 normal range; undo on the host after the run
        out_unscale = 1.0
        if base == "fp16":
            amax = float(np.abs(M).max())
            if amax > 0:
                e = int(np.floor(-np.log2(amax)))
                M = M * np.float32(2.0**e)
                out_unscale = 2.0**-e

        mmc = np.ascontiguousarray(
            M.reshape(KC, 128, D).transpose(1, 0, 2)
        ).astype(np_dt)
        xf = x.reshape(ROWS, D)
        in_maps = []
        for c in range(N_CORES):
            sh = xf[c * RPC : (c + 1) * RPC]               # [4096, 512]
            xT = sh.T.astype(np_dt)                        # [512, 4096]
            xr = xT.reshape(KC, 128, HB, HW)               # [k, p, h, c]
            xt0 = np.ascontiguousarray(xr[:, :, 0, :])     # [KC, 128, HW]
            xq = np.ascontiguousarray(
                xr[:, :, 1:, :].transpose(2, 1, 0, 3).reshape(HB - 1, 128, KC * HW)
            )
            in_maps.append({"xt0": xt0, "xq": xq, "mm": mmc})

    _last_in_maps = in_maps
    res = _run(in_maps, token)
    _last_result = res
    out = np.empty((ROWS, D), dtype=np.float32)
    for c in range(N_CORES):
        yb = res.results[c]["yt"].astype(np.float32)   # [HB, DT, 128, HW]
        if out_unscale != 1.0:
            yb *= np.float32(out_unscale)
        # yb[h, d, p, cc] = y[d*128+p, h*HW+cc] -> yc [512, 4096]
        yc = yb.transpose(1, 2, 0, 3).reshape(D, RPC)
        out[c * RPC : (c + 1) * RPC] = yc.T
    return out.reshape(B, N, D)


if __name__ == "__main__":
    # smoke test with random data
    rng = np.random.default_rng(0)
    x = rng.standard_normal((8, 4096, 512)).astype(np.float32)
    W_v = rng.standard_normal((512, 8, 64)).astype(np.float32) * 0.01
    s_p = np.ones((8,), np.float32)
    c_p = np.ones((8,), np.float32)
    W_p = rng.standard_normal((512, 8, 64)).astype(np.float32) * 0.01
    W_A = rng.standard_normal((256, 64)).astype(np.float32)
    W_o = rng.standard_normal((8, 64, 512)).astype(np.float32) * 0.01
    beta_p = rng.standard_normal((512,)).astype(np.float32) * 1e-5
    beta_i_p = rng.standard_normal((4096, 512)).astype(np.float32) * 1e-5
    out = kernel(x, W_v=W_v, s_p=s_p, c_p=c_p, W_p=W_p, W_A=W_A, W_o=W_o,
                 beta_p=beta_p, beta_i_p=beta_i_p)
    M = _fold_m(W_v, s_p, W_p, beta_p, W_o)
    exp = (x.reshape(-1, 512).astype(np.float64) @ M.astype(np.float64)).reshape(8, 4096, 512)
    err = np.abs(out - exp).max() / (np.abs(exp).max() + 1e-30)
    print("smoke rel err:", err)
